# revision 77
# baseline (speedup 1.0000x reference)
"""Trainium2 Bass kernel for BottleneckAttention (B=32, DIM=512, 4 heads,
dim_head=128, 32x32 spatial, N=1024).

Sharding: data-parallel over batch (4 batches per core x 8 cores).

Per-core pipeline (fp32r matmuls except the bf16 PV path; timing is
PE-column-bound, so every stage is organized around output columns):
  1. qkv projection vs host-transposed weights: Q^T,K^T [d,N] channel-major
     per head (Q pre-scaled by dh**-0.5); V [pixel,(h,d)] pixel-major, bf16
  2. rel-pos logits via small matmuls whose stationary weights are the
     host-gathered shifted tables RH[xq]/RW[yq] (rel->abs gather folded in)
     -> LHLW^T [64, heads, N]
  3. S^T[k,q] = K^T.T @ Qs^T (K=128) + E.T @ LHLW^T (K=64, E = one-hot
     expansion) accumulated per (k-chunk, half) in 1-bank PSUM tiles
  4. P^T = exp(S^T) on ScalarE, written bf16 (no max subtraction; logits
     are O(10))
  5. PV in [q, d] layout: per qblock, stationary P^T-chunk x moving V
     (bf16, 128-col matmuls run 1 cyc/row); denominators ride along as
     [q, 1] columns via a ones moving operand (~1 cycle each instead of a
     512-column M=1 matmul)
  6. normalize: DVE reciprocal on the [q-part, 8] denominator tile + one
     stride-0-broadcast tensor_mul per half; store [q, (h d)]-major, the
     host transposes to channel-major
  7. batches are software-pipelined: batch bb+1's projection/rel thunks are
     interleaved into batch bb's attention slots (the attention phase is
     ScalarE-exp-bound, projection is PE-bound, so they complement); the
     last head finishes in two qblock waves so half the normalize hides
     under the other wave's PV matmuls
"""
import numpy as np

import concourse.bass as bass
import concourse.bacc as bacc
import concourse.mybir as mybir
import concourse.tile as tile
from concourse import bass_utils

B, DIM, HEADS, DH, H, W = 32, 512, 4, 128, 32, 32
N = H * W
NCORES = 8
BPC = B // NCORES  # batches per core
SCALE = DH ** -0.5

F32 = mybir.dt.float32
F32R = mybir.dt.float32r
BF16 = mybir.dt.bfloat16
AF = mybir.ActivationFunctionType

_cached_nc = None


def _build_program():
    nc = bacc.Bacc("TRN2", target_bir_lowering=False, debug=False)

    x4 = nc.dram_tensor("x4", [BPC, 128, 4, N], F32, kind="ExternalInput").ap()
    wqk = nc.dram_tensor("wqk", [128, 4, 2 * HEADS * DH], F32, kind="ExternalInput").ap()
    wv = nc.dram_tensor("wv", [128, 4, HEADS * DH], F32, kind="ExternalInput").ap()
    rh = nc.dram_tensor("rh", [128, 32, 32], F32, kind="ExternalInput").ap()
    rw = nc.dram_tensor("rw", [128, 32, 32], F32, kind="ExternalInput").ap()
    em = nc.dram_tensor("em", [64, N], F32, kind="ExternalInput").ap()
    out4 = nc.dram_tensor("out4", [BPC, N, HEADS * DH], F32, kind="ExternalOutput").ap()

    HALF = slice(0, 512), slice(512, 1024)

    with tile.TileContext(nc) as tc:
        with tc.tile_pool(name="cpool", bufs=1) as cpool, \
             tc.tile_pool(name="xpool", bufs=8) as xpool, \
             tc.tile_pool(name="qkpool", bufs=2) as qkpool, \
             tc.tile_pool(name="vpool", bufs=2) as vpool, \
             tc.tile_pool(name="lhlwpool", bufs=2) as lhlwpool, \
             tc.tile_pool(name="ptpool", bufs=8) as ptpool, \
             tc.tile_pool(name="denpool", bufs=2) as denpool, \
             tc.tile_pool(name="outpool", bufs=2) as outpool, \
             tc.tile_pool(name="psA", bufs=3, space="PSUM") as psA, \
             tc.tile_pool(name="psP", bufs=2, space="PSUM") as psP, \
             tc.tile_pool(name="psO", bufs=1, space="PSUM") as psO, \
             tc.tile_pool(name="psD", bufs=1, space="PSUM") as psD:

            # ---- constants; DMA order matters: the first proj matmul needs
            #      wqk[cc0] + x(b0)[cc0], so those go first --------------------
            e_sb = cpool.tile([64, N], F32R)
            rh_sb = cpool.tile([128, 32, 32], F32R)
            rw_sb = cpool.tile([128, 32, 32], F32R)
            ones_bf = cpool.tile([128, 1], BF16)
            wqk_sb = cpool.tile([128, 4, 2 * HEADS * DH], F32R)
            wv_sb = cpool.tile([128, 4, HEADS * DH], F32R)
            nc.vector.memset(ones_bf, 1.0)

            prefetched_x = {}

            def load_x(bb):
                # batch 0 loads ride the idle ScalarE queue (parallel to the
                # weight DMAs on SP at startup); prefetches for later batches
                # go on SP, whose stream is nearly empty, so the triggers
                # fire as soon as the x slots free up (ScalarE is busy with
                # copies mid-batch and would fire them ~20us late).
                eng = nc.scalar if bb == 0 else nc.sync
                tiles = []
                for cc in range(4):
                    xt = xpool.tile([128, N], F32R, tag="x", name=f"x_{bb}_{cc}")
                    if bb == 0 and cc == 0:
                        # split so the first half lands early
                        eng.dma_start(out=xt[:, 0:512],
                                      in_=x4[bb, :, cc, 0:512].bitcast(F32R))
                        eng.dma_start(out=xt[:, 512:],
                                      in_=x4[bb, :, cc, 512:].bitcast(F32R))
                    else:
                        eng.dma_start(out=xt, in_=x4[bb, :, cc, :].bitcast(F32R))
                    tiles.append(xt)
                prefetched_x[bb] = tiles

            # consumption-ordered startup: batch-0 proj tile t needs ALL of
            # x(b0) plus wqk[:, cc, t*128:(t+1)*128]; interleave x chunks
            # with the first t-slices so PE ramps as transfers land (the sim
            # serializes all DMA on one device, so order == arrival order)
            xt0 = xpool.tile([128, N], F32R, tag="x", name="x_0_0")
            nc.scalar.dma_start(out=xt0[:, 0:512],
                                in_=x4[0, :, 0, 0:512].bitcast(F32R))
            nc.sync.dma_start(out=wqk_sb[:, 0, 0:256],
                              in_=wqk[:, 0, 0:256].bitcast(F32R))
            nc.scalar.dma_start(out=xt0[:, 512:],
                                in_=x4[0, :, 0, 512:].bitcast(F32R))
            xts = [xt0]
            for cc in range(1, 4):
                xt = xpool.tile([128, N], F32R, tag="x", name=f"x_0_{cc}")
                nc.scalar.dma_start(out=xt, in_=x4[0, :, cc, :].bitcast(F32R))
                nc.sync.dma_start(out=wqk_sb[:, cc, 0:256],
                                  in_=wqk[:, cc, 0:256].bitcast(F32R))
                xts.append(xt)
            prefetched_x[0] = xts
            # remaining weights in t-consumption order
            for ts0 in range(256, 2 * HEADS * DH, 256):
                for cc in range(4):
                    nc.sync.dma_start(
                        out=wqk_sb[:, cc, ts0:ts0 + 256],
                        in_=wqk[:, cc, ts0:ts0 + 256].bitcast(F32R))
            nc.sync.dma_start(out=rh_sb, in_=rh.bitcast(F32R))
            for cc in range(4):
                nc.sync.dma_start(out=wv_sb[:, cc, :], in_=wv[:, cc, :].bitcast(F32R))
            nc.sync.dma_start(out=rw_sb, in_=rw.bitcast(F32R))
            nc.sync.dma_start(out=e_sb, in_=em.bitcast(F32R))

            def make_batch_ctx(bb):
                x_cc = prefetched_x.pop(bb)
                # batch 0's projection runs upfront while psA (the s-tile
                # pool) is idle, so it gets the 3-slot pool; interleaved
                # projections use the dedicated 2-slot psP pool
                pspool = psA if bb == 0 else psP

                # ---- qkv projection ---------------------------------------
                # t = kk*HEADS + h; Q tiles (0-3) first so rel can start
                qk_sb = qkpool.tile([128, 8, N], F32R, tag="qk",
                                    name=f"qk_{bb}")
                def proj_qk(t):
                    # both halves per cc -> each weight chunk loaded once
                    pj0 = pspool.tile([128, 512], F32, tag="big",
                                      name=f"pj_{bb}_{t}_0")
                    pj1 = pspool.tile([128, 512], F32, tag="big",
                                      name=f"pj_{bb}_{t}_1")
                    for cc in range(4):
                        w = wqk_sb[:, cc, t * 128:(t + 1) * 128]
                        nc.tensor.matmul(pj0, w, x_cc[cc][:, HALF[0]],
                                         start=(cc == 0), stop=(cc == 3))
                        nc.tensor.matmul(pj1, w, x_cc[cc][:, HALF[1]],
                                         start=(cc == 0), stop=(cc == 3))
                    # psum->sbuf copies ride DVE when these thunks run
                    # inside an attention phase (ScalarE saturated by exps);
                    # batch 0 runs upfront where ScalarE is idle, so split
                    cp = nc.scalar.copy if bb == 0 else nc.vector.tensor_copy
                    cp(qk_sb[:, t, HALF[0]], pj0)
                    nc.vector.tensor_copy(qk_sb[:, t, HALF[1]], pj1)
                v_sb = vpool.tile([128, 8, HEADS * DH], BF16, tag="v",
                                  name=f"v_{bb}")
                def proj_v(pc):
                    pv = psP.tile([128, HEADS * DH], F32, tag="big",
                                  name=f"pvp_{bb}_{pc}")
                    for cc in range(4):
                        nc.tensor.matmul(
                            pv,
                            x_cc[cc][:, pc * 128:(pc + 1) * 128],
                            wv_sb[:, cc, :],
                            start=(cc == 0), stop=(cc == 3),
                        )
                    if bb == 0 and pc % 2 == 0:
                        nc.scalar.copy(v_sb[:, pc, :], pv)
                    else:
                        nc.vector.tensor_copy(v_sb[:, pc, :], pv)

                # ---- rel-pos logits ---------------------------------------
                # Pack (2 g-groups) x (4 heads) per matmul: stationary
                # [128, 64] = rh[:, g:g+2, :], moving 256 cols = (g2, t, yq).
                # N=256 keeps fp32r at 1 cyc/row (N<256 runs 4x slower).
                # Valid output: row-half rh <-> g2=rh; junk elsewhere ignored.
                qv4 = qk_sb[:, 0:4, :].rearrange("p t (a b) -> p t a b",
                                                 a=32, b=32)
                lhlw_all = lhlwpool.tile([64, HEADS, N], F32R, tag="lhlw",
                                         name=f"lhlw_{bb}")
                lhv = lhlw_all[0:32].rearrange("p t (a b) -> p t a b", a=32, b=32)
                lwv = lhlw_all[32:64].rearrange("p t (a b) -> p t a b", a=32, b=32)

                def rel_blk(tt, which):
                    tab = rh_sb if which == 0 else rw_sb
                    ps = pspool.tile([64, 512], F32, tag="big",
                                     name=f"rel_{bb}_{which}_{tt}")
                    for mm in range(2):
                        j = tt * 2 + mm
                        g0 = j * 2
                        lhsT = tab[:, g0:g0 + 2, :].rearrange("p a b -> p (a b)")
                        if which == 0:
                            # cols (g2, t, yq): qv4 dims [p, t, g2, yq]
                            rv = qv4[:, :, g0:g0 + 2, :]
                            rhs = bass.AP(tensor=rv.tensor, offset=rv.offset,
                                          ap=[rv.ap[0], rv.ap[2], rv.ap[1], rv.ap[3]])
                        else:
                            # cols (g2, t, xq): qv4 dims [p, t, xq, g2]
                            rv = qv4[:, :, :, g0:g0 + 2]
                            rhs = bass.AP(tensor=rv.tensor, offset=rv.offset,
                                          ap=[rv.ap[0], rv.ap[3], rv.ap[1], rv.ap[2]])
                        nc.tensor.matmul(ps[:, mm * 256:(mm + 1) * 256],
                                         lhsT, rhs, start=True, stop=True)
                    # psum cols: (mm, g2, t, c32); valid g2 == row-half
                    psv = ps.rearrange("p (m g t c) -> p m g t c", m=2, g=2, t=4)
                    for rh_i in range(2):
                        src = psv[rh_i * 32:(rh_i + 1) * 32, :, rh_i, :, :]
                        # src dims [p, mm, t, c]; reorder to [p, t, mm, c]
                        srct = bass.AP(tensor=src.tensor, offset=src.offset,
                                       ap=[src.ap[0], src.ap[2], src.ap[1], src.ap[3]])
                        # g = 4*tt + 2*mm + rh_i
                        if which == 0:
                            # dest xq=g: [p(xk), t, xq {step2}, yq 32]
                            d = lhv[:, :, (4 * tt + rh_i)::2, :]
                            d = bass.AP(tensor=d.tensor, offset=d.offset,
                                        ap=[d.ap[0], d.ap[1],
                                            [d.ap[2][0], 2], d.ap[3]])
                        else:
                            # dest yq=g: [p(yk), t, xq 32, yq {step2}]
                            d0 = lwv[:, :, :, (4 * tt + rh_i)::2]
                            d0 = bass.AP(tensor=d0.tensor, offset=d0.offset,
                                         ap=[d0.ap[0], d0.ap[1], d0.ap[2],
                                             [d0.ap[3][0], 2]])
                            # reorder dest dims to [p, t, yq2, xq]
                            d = bass.AP(tensor=d0.tensor, offset=d0.offset,
                                        ap=[d0.ap[0], d0.ap[1], d0.ap[3], d0.ap[2]])
                        # batch 0's rels run before attention, when ScalarE
                        # is idle: alternate the copies so the 3-slot psum
                        # rotation isn't gated on a serial DVE chain
                        if bb == 0 and (tt + rh_i) % 2 == 0:
                            nc.scalar.copy(d, srct)
                        else:
                            nc.vector.tensor_copy(d, srct)

                # thunk order: Q tiles first (rel needs them), lh blocks ride
                # the K tiles, lw blocks ride the V projection. 32 thunks ==
                # one per (head, kc) slot of the previous batch's attention.
                thunks = []
                for t in range(4):
                    thunks.append(lambda t=t: proj_qk(t))
                for t in (4, 5, 6, 7):
                    thunks.append(lambda t=t: proj_qk(t))
                    thunks.append(lambda a=2 * (t - 4): rel_blk(a, 0))
                    thunks.append(lambda a=2 * (t - 4) + 1: rel_blk(a, 0))
                # lw-rels before the V projections: the next batch's first
                # E-matmuls need the full lhlw, while V isn't consumed until
                # its pv(kc=0) three slots in
                nv = 4
                for pc in range(8):
                    thunks.append(lambda pc=pc: rel_blk(pc, 1))
                    if pc < nv:
                        thunks.append(lambda pc=pc: proj_v(pc))
                # the trailing V chunks aren't consumed until pv(kc>=0) of
                # the batch's own head 0 (3-slot lag), so they can run inside
                # its attention window -- filling slots the (shorter)
                # next-batch thunk list leaves empty; for batch 0 ALL V moves
                # there, trimming the DMA-bound upfront phase
                vdefer = [lambda pc=pc: proj_v(pc) for pc in range(nv, 8)]

                return dict(qk_sb=qk_sb, v_sb=v_sb, lhlw_all=lhlw_all,
                            thunks=thunks, vdefer=vdefer)

            # batch 0's projection runs upfront (nothing to overlap with)
            ctx = make_batch_ctx(0)
            for th in ctx["thunks"]:
                th()
            ctx["thunks"] = []
            load_x(1)

            for bb in range(BPC):
                nxt = make_batch_ctx(bb + 1) if bb + 1 < BPC else None
                if bb + 2 < BPC:
                    load_x(bb + 2)
                qk_sb = ctx["qk_sb"]
                v_sb = ctx["v_sb"]
                lhlw_all = ctx["lhlw_all"]
                thunks = list(ctx["vdefer"]) + (nxt["thunks"] if nxt else [])
                tpos = [0]

                def pop_thunk():
                    if tpos[0] < len(thunks):
                        thunks[tpos[0]]()
                        tpos[0] += 1

                slot = [0]
                nslots = HEADS * 8
                for h in range(HEADS):
                    qs = qk_sb[:, h, :]
                    ks = qk_sb[:, HEADS + h, :]
                    lhlw_sb = lhlw_all[:, h, :]

                    # out in [q, d] layout (one [128, 128] tile per qblock);
                    # host transposes. bf16 P/V keep the ap-128 PV matmuls at
                    # 1 cyc/row (fp32r under 256 cols runs 4x slower).
                    outp = psO.tile([128, 8, DH], F32, tag="out")
                    denq = psD.tile([128, 8], F32, tag="denq")
                    pt_l = [None] * 8

                    def emit_s(kc):
                        kchunk = slice(kc * 128, (kc + 1) * 128)
                        pt = ptpool.tile([128, N], BF16, tag="pt", name=f"pt_{kc}")
                        pt_l[kc] = pt
                        # K stationary loaded once (both halves), then E
                        # stationary once; exp(half) right after its E-mm
                        # the last batch has no proj thunks, so psP is idle:
                        # use it for every other s-tile to deepen the buffer
                        sp1 = psP if bb == BPC - 1 else psA
                        s0 = psA.tile([128, 512], F32, tag="big", name=f"sps_{kc}_0")
                        s1 = sp1.tile([128, 512], F32, tag="big", name=f"sps_{kc}_1")
                        nc.tensor.matmul(s0, ks[:, kchunk], qs[:, HALF[0]],
                                         start=True, stop=False)
                        nc.tensor.matmul(s1, ks[:, kchunk], qs[:, HALF[1]],
                                         start=True, stop=False)
                        nc.tensor.matmul(s0, e_sb[:, kchunk], lhlw_sb[:, HALF[0]],
                                         start=False, stop=True)
                        nc.scalar.activation(out=pt[:, HALF[0]], in_=s0,
                                             func=AF.Exp)
                        nc.tensor.matmul(s1, e_sb[:, kchunk], lhlw_sb[:, HALF[1]],
                                         start=False, stop=True)
                        nc.scalar.activation(out=pt[:, HALF[1]], in_=s1,
                                             func=AF.Exp)

                    def emit_pv(kc, qbs=tuple(range(8)), dent=None, doff=0):
                        # psum zero-regions are whole 2KB banks: only the
                        # first write into a bank may carry start=True (it
                        # zeroes the full region) and only the last write
                        # carries stop
                        dent = denq if dent is None else dent
                        pt = pt_l[kc]
                        vs = v_sb[:, kc, h * DH:(h + 1) * DH]
                        for qb in qbs:
                            nc.tensor.matmul(outp[:, qb, :],
                                             pt[:, qb * 128:(qb + 1) * 128],
                                             vs,
                                             start=(kc == 0 and qb % 4 == 0),
                                             stop=(kc == 7 and qb % 4 == 3))
                        # denominators ride as [q, 1] columns: pt is the
                        # (free) stationary operand, ones the 1-col moving
                        # one, so each sum costs ~1 PE cycle instead of 512
                        for qb in qbs:
                            dq = qb - doff
                            nc.tensor.matmul(dent[:, dq:dq + 1],
                                             pt[:, qb * 128:(qb + 1) * 128],
                                             ones_bf,
                                             start=(kc == 0 and qb == qbs[0]),
                                             stop=(kc == 7 and qb == qbs[-1]))

                    def rbcast(rden, sl):
                        # reciprocal column per qblock, broadcast along d via
                        # a stride-0 free level
                        rv = rden[:, sl]
                        return bass.AP(tensor=rv.tensor, offset=rv.offset,
                                       ap=[rv.ap[0], rv.ap[1], [0, DH]])

                    def emit_norm(bb=bb, h=h):
                        rden = denpool.tile([128, 8], F32, tag="rden")
                        out_sb = outpool.tile([128, 8, DH], F32, tag="osb")
                        nc.vector.reciprocal(rden, denq)
                        for qp in range(2):
                            sl = slice(qp * 4, qp * 4 + 4)
                            nc.vector.tensor_mul(out_sb[:, sl, :],
                                                 outp[:, sl, :],
                                                 rbcast(rden, sl))
                            dst = out4[bb, qp * 512:(qp + 1) * 512,
                                       h * DH:(h + 1) * DH]
                            dst = dst.rearrange("(a q) d -> q a d", a=4)
                            nc.sync.dma_start(out=dst, in_=out_sb[:, sl, :])

                    # PV lags S by 3 kc-iterations: the first pv of a head
                    # must wait for the previous head's normalize to release
                    # the psO tiles (bufs=1), so give it ~1.7us of S-work
                    # cover (pt pool holds 4 chunks).
                    last = (bb == BPC - 1) and (h == HEADS - 1)

                    if not last:
                        for kc in range(8):
                            emit_s(kc)
                            if kc >= 3:
                                emit_pv(kc - 3)
                            pop_thunk()
                            slot[0] += 1
                            # catch up when the thunk list outnumbers the
                            # remaining slots (batch 0 defers all 8 V projs)
                            if len(thunks) - tpos[0] > nslots - slot[0]:
                                pop_thunk()
                        for kc in (5, 6):
                            emit_pv(kc)
                        emit_pv(7)
                        emit_norm()
                    else:
                        # ---- last head: two-wave finish to shrink the tail.
                        # Wave A (qblocks 0-3) completes first; its
                        # normalize+store runs while the PE crunches wave B's
                        # (qblocks 4-7) PV matmuls, so only half the epilogue
                        # trails the last matmul. Needs all 8 pt chunks live.
                        wA, wB = (0, 1, 2, 3), (4, 5, 6, 7)
                        for kc in range(8):
                            emit_s(kc)
                            if kc >= 3:
                                emit_pv(kc - 3, wA)
                        for kc in (5, 6, 7):
                            emit_pv(kc, wA)
                        rden = denpool.tile([128, 8], F32, tag="rden")
                        out_sb = outpool.tile([128, 8, DH], F32, tag="osb")
                        nc.vector.reciprocal(rden[:, 0:4], denq[:, 0:4])
                        nc.vector.tensor_mul(out_sb[:, 0:4, :],
                                             outp[:, 0:4, :],
                                             rbcast(rden, slice(0, 4)))
                        dstA = out4[bb, 0:512, h * DH:(h + 1) * DH]
                        nc.sync.dma_start(
                            out=dstA.rearrange("(a q) d -> q a d", a=4),
                            in_=out_sb[:, 0:4, :])
                        # wave B denominators first (reciprocal hides under
                        # the PV finals); outs in two psP slots so the first
                        # store pair issues while the second half finishes
                        denqB = psD.tile([128, 4], F32, tag="denq")
                        for kc in range(8):
                            pt = pt_l[kc]
                            for qb in wB:
                                nc.tensor.matmul(
                                    denqB[:, qb - 4:qb - 3],
                                    pt[:, qb * 128:(qb + 1) * 128], ones_bf,
                                    start=(kc == 0 and qb == 4),
                                    stop=(kc == 7 and qb == 7))
                        nc.vector.reciprocal(rden[:, 4:8], denqB)
                        for qp in (0, 1):
                            outpB = psP.tile([128, 2, DH], F32, tag="big",
                                             name=f"outpB_{qp}")
                            qbs = wB[qp * 2:qp * 2 + 2]
                            for kc in range(8):
                                pt = pt_l[kc]
                                vs = v_sb[:, kc, h * DH:(h + 1) * DH]
                                for qb in qbs:
                                    nc.tensor.matmul(
                                        outpB[:, qb - qbs[0], :],
                                        pt[:, qb * 128:(qb + 1) * 128], vs,
                                        start=(kc == 0 and qb == qbs[0]),
                                        stop=(kc == 7 and qb == qbs[-1]))
                            sl = slice(4 + qp * 2, 6 + qp * 2)
                            nc.vector.tensor_mul(out_sb[:, sl, :], outpB,
                                                 rbcast(rden, sl))
                            q0 = (4 + qp * 2) * 128
                            dstB = out4[bb, q0:q0 + 256, h * DH:(h + 1) * DH]
                            nc.sync.dma_start(
                                out=dstB.rearrange("(a q) d -> q a d", a=2),
                                in_=out_sb[:, sl, :])

                while tpos[0] < len(thunks):
                    pop_thunk()
                ctx = nxt

    nc.compile()
    return nc


def _get_program():
    global _cached_nc
    if _cached_nc is None:
        _cached_nc = _build_program()
    return _cached_nc


def _prep_inputs(x, w_qkv, rel_h, rel_w):
    x = np.ascontiguousarray(x, dtype=np.float32)
    w_qkv = np.asarray(w_qkv, dtype=np.float32)
    rel_h = np.asarray(rel_h, dtype=np.float32)
    rel_w = np.asarray(rel_w, dtype=np.float32)

    # x: (B, 512, 32, 32) -> (B, 128, 4, N) with c = cc*128 + p
    x_in = np.ascontiguousarray(
        x.reshape(B, 4, 128, N).transpose(0, 2, 1, 3))

    # w_qkv rows are channels o = d*12 + k*4 + h
    w3 = w_qkv.reshape(DH, 3, HEADS, DIM)         # [d, k, h, c]
    wq_chd = np.transpose(w3[:, 0], (2, 1, 0))    # [c, h, d]
    wk_chd = np.transpose(w3[:, 1], (2, 1, 0))
    wv_chd = np.transpose(w3[:, 2], (2, 1, 0))
    wqk_full = np.concatenate([
        (wq_chd * SCALE).reshape(DIM, HEADS * DH),
        wk_chd.reshape(DIM, HEADS * DH),
    ], axis=1)                                    # [512, 1024]
    wqk_in = np.ascontiguousarray(
        wqk_full.reshape(4, 128, 2 * HEADS * DH).transpose(1, 0, 2))
    wv_in = np.ascontiguousarray(
        wv_chd.reshape(DIM, HEADS * DH).reshape(4, 128, HEADS * DH).transpose(1, 0, 2))

    # shifted rel tables; divide by SCALE because Q is pre-scaled
    idx = np.arange(32)[None, :] - np.arange(32)[:, None] + 31   # [q, k]
    rh_in = np.ascontiguousarray(
        np.transpose(rel_h[idx] / SCALE, (2, 0, 1)))  # [d, xq, xk]
    rw_in = np.ascontiguousarray(
        np.transpose(rel_w[idx] / SCALE, (2, 0, 1)))  # [d, yq, yk]

    # E: [64, N]; rows 0:32 select xk, rows 32:64 select yk
    eye = np.eye(32, dtype=np.float32)
    em_in = np.concatenate([
        np.kron(eye, np.ones((1, 32), dtype=np.float32)),
        np.tile(eye, (1, 32)),
    ], axis=0)

    in_maps = []
    for c in range(NCORES):
        in_maps.append({
            "x4": x_in[c * BPC:(c + 1) * BPC],
            "wqk": wqk_in,
            "wv": wv_in,
            "rh": rh_in,
            "rw": rw_in,
            "em": em_in,
        })
    return in_maps


def run(inputs, trace=False):
    nc = _get_program()
    in_maps = _prep_inputs(**inputs)
    res = bass_utils.run_bass_kernel_spmd(
        nc, in_maps, core_ids=list(range(NCORES)), trace=trace)
    parts = [res.results[c]["out4"].transpose(0, 2, 1).reshape(
                 BPC, HEADS * DH, H, W)
             for c in range(NCORES)]
    out = np.concatenate(parts, axis=0).astype(np.float32)
    return out, res


def kernel(x, w_qkv, rel_h, rel_w):
    out, _ = run(dict(x=x, w_qkv=w_qkv, rel_h=rel_h, rel_w=rel_w))
    return out

